# revision 27
# baseline (speedup 1.0000x reference)
"""Trainium2 Bass kernel for a pre-LN transformer decoder layer.

Shapes (hardcoded): B=2, S=2048, D=1024, H=16, DH=64, F=4096.
Returns (x_out [B,S,D], att [B,H,S,S]) like the reference.

Sharding: 8 cores; core c -> batch b=c//4, j=c%4 owns the four 128-row
query tiles {j, 7-j, 8+j, 15-j} of its batch.  Sorted by causal extent
these tiles need exactly {1,2,3,4} p-blocks of 512 keys, so every core
runs the identical (SPMD) program; which rows/mask a core gets is pure
data.  Each core computes LN1+K/V over the full 2048 rows of its batch
(duplicated across the 4 cores of a batch - keeps the program uniform
and the cores perfectly load balanced), Q/attention/W_O/MLP only for
its own 512 query rows.

Layout: activations are kept feature-on-partitions ("transposed") via
PE-transpose right after LN, so every projection is a plain
lhsT[K=feat,M] x rhs[K=feat,N=rows] matmul.  Scores are computed
transposed ([p, q]); softmax skips max-subtraction (scaled scores are
O(1); the -1.25e9 causal bias underflows exp to exactly 0) and the
denominator rides the A@V matmul as an appended ones-column of V.
Attention internals run in bf16, projections in float32r (full PE rate
at free-dim >= 256).
"""

import numpy as np

B, S, D, H, DH, F = 2, 2048, 1024, 16, 64, 4096
P = 128
DC = D // P          # 8 feature chunks
RC = S // P          # 16 row chunks per batch
RB = S // 512        # 4 row blocks of 512
QT = 4               # q tiles per core
QW = QT * P          # 512 query rows per core
HP = H // 2          # 8 head pairs
NCORES = 8
MASK_NEG = -1.25e9   # -1e10 / sqrt(DH)

_CACHE = {}


def _build_program():
    import concourse.bass as bass
    import concourse.mybir as mybir
    import concourse.tile as tile
    from concourse import bacc
    from concourse.masks import make_identity

    f32 = mybir.dt.float32
    f32r = mybir.dt.float32r
    bf16 = mybir.dt.bfloat16
    AF = mybir.ActivationFunctionType
    OP = mybir.AluOpType

    nc = bacc.Bacc(
        "TRN2", target_bir_lowering=False, debug=False, num_devices=NCORES
    )

    def din(name, shape, dt=None):
        return nc.dram_tensor(name, list(shape), dt or f32,
                              kind="ExternalInput").ap()

    def dout(name, shape):
        return nc.dram_tensor(name, list(shape), f32, kind="ExternalOutput").ap()

    x_full = din("x_full", [S, D])
    x_q = din("x_q", [QW, D])
    wq_m = din("wq_m", [D, H * DH], bf16)     # [d, h*64+dh], prescaled by 1/8
    wk_m = din("wk_m", [D, H * DH], bf16)
    wv_m = din("wv_m", [D, H * DH], bf16)
    wo_m = din("wo_m", [H * DH, D], bf16)
    w1_m = din("w1_m", [D, F], bf16)
    w2_m = din("w2_m", [F, D], bf16)
    ln1w = din("ln1w", [D])
    ln1b = din("ln1b", [D])
    ln2w = din("ln2w", [D])
    ln2b = din("ln2b", [D])
    b1v = din("b1v", [F])
    b2v = din("b2v", [D])
    maskd = din("maskd", [QT, 4, P, P], bf16)  # [i,c,p,q] 0/1 multiplicative

    att_t = nc.dram_tensor("att_t", [H, S, QW], bf16,
                           kind="ExternalOutput").ap()  # [h, p, q] bf16
    out_t = dout("out_t", [D, QW])       # transposed output [d, q]

    with tile.TileContext(nc, pool_alloc_mode="queue") as tc:
        _body(tc, nc, locals())
    nc.compile()
    return nc


def _body(tc, nc, t):
    import concourse.mybir as mybir
    from concourse.masks import make_identity

    f32 = mybir.dt.float32
    f32r = mybir.dt.float32r
    bf16 = mybir.dt.bfloat16
    AF = mybir.ActivationFunctionType
    OP = mybir.AluOpType
    X = mybir.AxisListType

    x_full, x_q = t["x_full"], t["x_q"]
    wq_m, wk_m, wv_m, wo_m = t["wq_m"], t["wk_m"], t["wv_m"], t["wo_m"]
    w1_m, w2_m = t["w1_m"], t["w2_m"]
    ln1w, ln1b, ln2w, ln2b = t["ln1w"], t["ln1b"], t["ln2w"], t["ln2b"]
    b1v, b2v, maskd = t["b1v"], t["b2v"], t["maskd"]
    att_t, out_t = t["att_t"], t["out_t"]

    r32 = lambda ap: ap.bitcast(f32r)

    const = tc.alloc_tile_pool(name="const", bufs=1)

    ident = const.tile([P, P], f32, tag="ident")
    make_identity(nc, ident)
    ones_col = const.tile([P, 1], f32, tag="ones_col")
    nc.vector.memset(ones_col, 1.0)
    ones_row = const.tile([P, P], f32, tag="ones_row")
    nc.vector.memset(ones_row, 1.0)
    eps1 = const.tile([P, 1], f32, tag="eps1")
    nc.vector.memset(eps1, 1e-5)

    def load_vec_chunks(ap_dram, name):
        # [D] -> [128, DC] (column dc = chunk dc, per-partition scalars)
        tile_ = const.tile([P, D // P], f32, tag=name)
        nc.sync.dma_start(
            out=tile_, in_=ap_dram.rearrange("(c p) -> p c", p=P)
        )
        return tile_

    ln1w_sb = load_vec_chunks(ln1w, "ln1w_sb")
    ln1b_sb = load_vec_chunks(ln1b, "ln1b_sb")
    ln2w_sb = load_vec_chunks(ln2w, "ln2w_sb")
    ln2b_sb = load_vec_chunks(ln2b, "ln2b_sb")
    b2_sb = load_vec_chunks(b2v, "b2_sb")
    b1_sb = const.tile([P, F // P], f32, tag="b1_sb")
    nc.sync.dma_start(out=b1_sb, in_=b1v.rearrange("(c p) -> p c", p=P))

    mask_sb = const.tile([P, QT, 4, P], bf16, tag="mask_sb")
    nc.sync.dma_start(out=mask_sb, in_=maskd.rearrange("i c p q -> p i c q"))

    # ---------------- Phase 1: LN1 + transpose of x_full -> yT ----------
    yT_pool = tc.alloc_tile_pool(name="yT_pool", bufs=1, side="right")
    yT = [yT_pool.tile([P, S], bf16, tag=f"yT{dc}", name=f"yT{dc}") for dc in range(DC)]

    def ln_transpose(xin_pool, stat_pool, tpsum_pool, src_dram, r0, n_yt, yt_col,
                     also_xt=None):
        # Load 128 rows starting at r0, LN them, write transposed chunks into
        # n_yt[dc][:, yt_col:yt_col+128]; optionally also raw-transpose into
        # also_xt[dc][:, ...].
        xt_ = xin_pool.tile([P, D], f32, tag="x_in")
        nc.sync.dma_start(out=xt_, in_=src_dram[r0 : r0 + P, :])
        stats = stat_pool.tile([P, 2, 6], f32, tag="stats")
        for sg in range(2):
            nc.vector.bn_stats(out=stats[:, sg, :], in_=xt_[:, sg * 512 : (sg + 1) * 512])
        mv = stat_pool.tile([P, 2], f32, tag="mv")
        nc.vector.bn_aggr(out=mv, in_=stats)
        rstd = stat_pool.tile([P, 1], f32, tag="rstd")
        nc.scalar.activation(out=rstd, in_=mv[:, 1:2], func=AF.Sqrt,
                             bias=eps1, scale=1.0)
        nc.vector.reciprocal(out=rstd, in_=rstd)
        n_t = xin_pool.tile([P, D], f32, tag="n_t")
        nc.vector.tensor_scalar(out=n_t, in0=xt_, scalar1=mv[:, 0:1],
                                scalar2=rstd, op0=OP.subtract, op1=OP.mult)
        for dc in range(DC):
            tp = tpsum_pool.tile([P, P], f32, tag="tp")
            nc.tensor.transpose(tp, n_t[:, dc * P : (dc + 1) * P], ident)
            nc.scalar.activation(
                out=n_yt[dc][:, yt_col : yt_col + P], in_=tp, func=AF.Identity,
                scale=ln1w_sb[:, dc : dc + 1], bias=ln1b_sb[:, dc : dc + 1])
            if also_xt is not None:
                tp2 = tpsum_pool.tile([P, P], f32, tag="tp2")
                nc.tensor.transpose(tp2, xt_[:, dc * P : (dc + 1) * P], ident)
                nc.vector.tensor_copy(out=also_xt[dc][:, yt_col : yt_col + P],
                                      in_=tp2)

    with tc.tile_pool(name="ph1", bufs=3) as xin_pool, \
         tc.tile_pool(name="ph1s", bufs=4) as stat_pool, \
         tc.tile_pool(name="ph1p", bufs=4, space="PSUM") as tpsum_pool:
        for rc in range(RC):
            ln_transpose(xin_pool, stat_pool, tpsum_pool, x_full, rc * P,
                         yT, rc * P)

    # ---------------- Phase 2: K projection -> kT (bf16) ----------------
    attin_pool = tc.alloc_tile_pool(name="attin_pool", bufs=1)
    kT = [attin_pool.tile([P, S], bf16, tag=f"kT{hp}", name=f"kT{hp}") for hp in range(HP)]
    with tc.tile_pool(name="wk", bufs=1) as wkp, \
         tc.tile_pool(name="kps", bufs=4, space="PSUM") as kps:
        wk_sb = [wkp.tile([P, H * DH], bf16, tag=f"wk{dc}", name=f"wk{dc}") for dc in range(DC)]
        for dc in range(DC):
            nc.sync.dma_start(out=wk_sb[dc], in_=wk_m[dc * P : (dc + 1) * P, :])
        for hp in range(HP):
            for rb in range(RB):
                ps = kps.tile([P, 512], f32, tag="psK")
                for dc in range(DC):
                    nc.tensor.matmul(
                        ps, wk_sb[dc][:, hp * P : (hp + 1) * P],
                        yT[dc][:, rb * 512 : (rb + 1) * 512],
                        start=(dc == 0), stop=(dc == DC - 1))
                nc.scalar.activation(out=kT[hp][:, rb * 512 : (rb + 1) * 512],
                                     in_=ps, func=AF.Copy)

    # ---------------- Phase 3: V projection -> v_sb (bf16, ones col) ----
    # v_sb[rc] is [128, H*65]; column h*65+64 holds ones (denominator trick)
    v_sb = [attin_pool.tile([P, H * 65], bf16, tag=f"v{rc}", name=f"v{rc}") for rc in range(RC)]
    with tc.tile_pool(name="wv", bufs=1) as wvp, \
         tc.tile_pool(name="vps", bufs=4, space="PSUM") as vps:
        wv_sb = [wvp.tile([P, H * DH], bf16, tag=f"wv{dc}", name=f"wv{dc}") for dc in range(DC)]
        for dc in range(DC):
            nc.sync.dma_start(out=wv_sb[dc], in_=wv_m[dc * P : (dc + 1) * P, :])
        for rc in range(RC):
            nc.vector.memset(v_sb[rc], 1.0)
            for hg in range(2):
                ps = vps.tile([P, 512], f32, tag="psV")
                for dc in range(DC):
                    nc.tensor.matmul(
                        ps, yT[dc][:, rc * P : (rc + 1) * P],
                        wv_sb[dc][:, hg * 512 : (hg + 1) * 512],
                        start=(dc == 0), stop=(dc == DC - 1))
                vdst = v_sb[rc].rearrange("p (h e) -> p h e", e=65)
                nc.vector.tensor_copy(
                    out=vdst[:, hg * 8 : (hg + 1) * 8, 0:64],
                    in_=ps.rearrange("p (h e) -> p h e", e=64))

    yT_pool.release()

    # ---------------- Phase 4: x_q LN + transposes, Q projection --------
    xT_pool = tc.alloc_tile_pool(name="xT_pool", bufs=1, side="right")
    yqT_pool = tc.alloc_tile_pool(name="yqT_pool", bufs=1, side="right")
    yqT = [yqT_pool.tile([P, QW], bf16, tag=f"yqT{dc}", name=f"yqT{dc}") for dc in range(DC)]
    xT = [xT_pool.tile([P, QW], f32, tag=f"xT{dc}", name=f"xT{dc}") for dc in range(DC)]
    with tc.tile_pool(name="ph4", bufs=3) as xin_pool, \
         tc.tile_pool(name="ph4s", bufs=4) as stat_pool, \
         tc.tile_pool(name="ph4p", bufs=3, space="PSUM") as tpsum_pool:
        for qt in range(QT):
            ln_transpose(xin_pool, stat_pool, tpsum_pool, x_q, qt * P,
                         yqT, qt * P, also_xt=xT)

    qT = [attin_pool.tile([P, QW], bf16, tag=f"qT{hp}", name=f"qT{hp}") for hp in range(HP)]
    with tc.tile_pool(name="wq", bufs=1) as wqp, \
         tc.tile_pool(name="qps", bufs=4, space="PSUM") as qps:
        wq_sb = [wqp.tile([P, H * DH], bf16, tag=f"wq{dc}", name=f"wq{dc}") for dc in range(DC)]
        for dc in range(DC):
            nc.sync.dma_start(out=wq_sb[dc], in_=wq_m[dc * P : (dc + 1) * P, :])
        for hp in range(HP):
            ps = qps.tile([P, QW], f32, tag="psQ")
            for dc in range(DC):
                nc.tensor.matmul(ps, wq_sb[dc][:, hp * P : (hp + 1) * P],
                                 yqT[dc], start=(dc == 0),
                                 stop=(dc == DC - 1))
            nc.scalar.activation(out=qT[hp], in_=ps, func=AF.Copy)

    yqT_pool.release()

    # ---------------- Phase 5: attention ------------------------------
    z_pool = tc.alloc_tile_pool(name="z_pool", bufs=1, side="right")
    z_sb = [z_pool.tile([P, QW], bf16, tag=f"z{hp}", name=f"z{hp}") for hp in range(HP)]
    with tc.tile_pool(name="att_sp", bufs=3, space="PSUM") as spp, \
         tc.tile_pool(name="att_zp", bufs=2, space="PSUM") as zpp, \
         tc.tile_pool(name="att_exp", bufs=3) as expp, \
         tc.tile_pool(name="att_sm", bufs=3) as smp, \
         tc.tile_pool(name="att_dscr", bufs=3, space="DRAM") as dscr, \
         tc.tile_pool(name="att_st0", bufs=2) as attst0, \
         tc.tile_pool(name="att_st1", bufs=2) as attst1, \
         tc.tile_pool(name="att_st2", bufs=2) as attst2, \
         tc.tile_pool(name="att_st3", bufs=2) as attst3:
        attst = [attst0, attst1, attst2, attst3]
        for h in range(H):
            hp, hl = h // 2, h % 2
            base = hl * 64
            exps = []
            for i in range(QT):
                qcols = QW - 128 * i
                et = expp.tile([P, 4, qcols], bf16, tag=f"exp{i}", name=f"exp_h{h}_{i}")
                exps.append(et)
                if i < 2:
                    # one psum tile + exp per 128-p chunk
                    for c in range(4):
                        ps = spp.tile([P, qcols], f32, tag=f"sp{hl}")
                        nc.tensor.matmul(
                            ps,
                            kT[hp][base : base + 64,
                                   512 * i + 128 * c : 512 * i + 128 * (c + 1)],
                            qT[hp][base : base + 64, 128 * i : QW],
                            start=True, stop=True)
                        nc.scalar.activation(out=et[:, c, :], in_=ps,
                                             func=AF.Exp)
                else:
                    # pack chunks into one psum bank, single exp per group
                    ng = 2 if i == 2 else 4
                    for g in range(4 // ng):
                        ps = spp.tile([P, ng, qcols], f32, tag=f"sp{hl}")
                        for cc in range(ng):
                            c = g * ng + cc
                            nc.tensor.matmul(
                                ps[:, cc, :],
                                kT[hp][base : base + 64,
                                       512 * i + 128 * c : 512 * i + 128 * (c + 1)],
                                qT[hp][base : base + 64, 128 * i : QW],
                                start=True, stop=True)
                        nc.scalar.activation(
                            out=et[:, g * ng : (g + 1) * ng, :], in_=ps,
                            func=AF.Exp)
                # causal mask for this q-tile's own columns (batched, 4 chunks)
                nc.vector.tensor_tensor(
                    out=et[:, :, 0:P], in0=et[:, :, 0:P],
                    in1=mask_sb[:, i, :, :], op=OP.mult)
            zb = zpp.tile([65, QW], f32, tag="zb")
            for i in range(QT):
                for c in range(4):
                    nc.tensor.matmul(
                        zb[0:65, 128 * i : QW],
                        v_sb[4 * i + c][:, h * 65 : h * 65 + 65],
                        exps[i][:, c, :],
                        start=(i == 0 and c == 0),
                        stop=(i == QT - 1 and c == 3),
                        skip_group_check=True)
            rec64 = smp.tile([P, QW], bf16, tag="rec64")
            with nc.allow_low_precision(reason="bf16 attention path"):
                nc.vector.reciprocal(out=rec64[64:65, :], in_=zb[64:65, :])
            rec_d = dscr.tile([1, QW], bf16, tag="rec_d")
            nc.sync.dma_start(out=rec_d, in_=rec64[64:65, :])
            rec_bc = smp.tile([P, QW], bf16, tag="rec_bc")
            nc.sync.dma_start(out=rec_bc, in_=rec_d.to_broadcast([P, QW]))
            # z rows for this head: normalize straight out of PSUM
            zt = smp.tile([64, QW], bf16, tag="zt")
            nc.vector.tensor_tensor(out=zt, in0=zb[0:64, :],
                                    in1=rec_bc[0:64, :], op=OP.mult)
            nc.sync.dma_start(out=z_sb[hp][base : base + 64, :], in_=zt)
            # normalized attention out (f32), one batched mul + DMA per i
            for i in range(QT):
                qcols = QW - 128 * i
                af = attst[i].tile([P, 4, qcols], bf16, tag=f"af{i}",
                                   name=f"af{i}")
                nc.gpsimd.tensor_tensor(
                    out=af, in0=exps[i],
                    in1=rec_bc[:, 128 * i : QW].unsqueeze(1).to_broadcast(
                        [P, 4, qcols]),
                    op=OP.mult)
                nc.sync.dma_start(
                    out=att_t[h, 512 * i : 512 * (i + 1),
                              128 * i : QW].rearrange("(c p) q -> p c q", p=P),
                    in_=af)

    attin_pool.release()

    # ---------------- Phase 6: W_O, residual, LN2 -----------------------
    x2_pool = tc.alloc_tile_pool(name="x2_pool", bufs=1)
    x2T = [x2_pool.tile([P, QW], f32, tag=f"x2T{dc}", name=f"x2T{dc}") for dc in range(DC)]
    y2T = [x2_pool.tile([P, QW], bf16, tag=f"y2T{dc}", name=f"y2T{dc}") for dc in range(DC)]
    with tc.tile_pool(name="wo", bufs=1) as wop, \
         tc.tile_pool(name="ops", bufs=4, space="PSUM") as ops_, \
         tc.tile_pool(name="ln2", bufs=1) as ln2p:
        wo_sb = [wop.tile([P, D], bf16, tag=f"wo{kc}", name=f"wo{kc}") for kc in range(DC)]
        for kc in range(DC):
            nc.sync.dma_start(out=wo_sb[kc], in_=wo_m[kc * P : (kc + 1) * P, :])
        sq = [ln2p.tile([P, QW], f32, tag=f"sq{dc % 4}", name=f"sq{dc}") for dc in range(DC)]
        for dc in range(DC):
            ps = ops_.tile([P, QW], f32, tag="psO")
            for kc in range(DC):
                nc.tensor.matmul(ps, wo_sb[kc][:, dc * P : (dc + 1) * P],
                                 z_sb[kc], start=(kc == 0),
                                 stop=(kc == DC - 1))
            nc.vector.tensor_tensor(out=x2T[dc], in0=ps, in1=xT[dc], op=OP.add)
            nc.vector.tensor_tensor(out=sq[dc], in0=x2T[dc], in1=x2T[dc],
                                    op=OP.mult)
        ps1 = ops_.tile([1, QW], f32, tag="ps1", bufs=1)
        ps2 = ops_.tile([1, QW], f32, tag="ps2", bufs=1)
        for dc in range(DC):
            nc.tensor.matmul(ps1, ones_col, x2T[dc],
                             start=(dc == 0), stop=(dc == DC - 1))
        for dc in range(DC):
            nc.tensor.matmul(ps2, ones_col, sq[dc],
                             start=(dc == 0), stop=(dc == DC - 1))
        mu = ln2p.tile([1, QW], f32, tag="mu")
        nc.scalar.mul(out=mu, in_=ps1, mul=1.0 / D)
        ex2 = ln2p.tile([1, QW], f32, tag="ex2")
        nc.scalar.mul(out=ex2, in_=ps2, mul=1.0 / D)
        var = ln2p.tile([1, QW], f32, tag="var")
        nc.vector.tensor_tensor(out=var, in0=mu, in1=mu, op=OP.mult)
        nc.vector.tensor_tensor(out=var, in0=ex2, in1=var, op=OP.subtract)
        rstd2 = ln2p.tile([1, QW], f32, tag="rstd2")
        nc.scalar.activation(out=rstd2, in_=var, func=AF.Sqrt,
                             bias=eps1[0:1, :], scale=1.0)
        nc.vector.reciprocal(out=rstd2, in_=rstd2)
        mu_ps = ops_.tile([P, QW], f32, tag="mu_ps", bufs=1)
        nc.tensor.matmul(mu_ps, ones_row[0:1, :], mu, start=True, stop=True)
        mu_bc = ln2p.tile([P, QW], f32, tag="mu_bc")
        nc.scalar.activation(out=mu_bc, in_=mu_ps, func=AF.Copy)
        rstd_ps = ops_.tile([P, QW], f32, tag="rstd_ps", bufs=1)
        nc.tensor.matmul(rstd_ps, ones_row[0:1, :], rstd2, start=True, stop=True)
        rstd_bc = ln2p.tile([P, QW], f32, tag="rstd_bc")
        nc.scalar.activation(out=rstd_bc, in_=rstd_ps, func=AF.Copy)
        for dc in range(DC):
            tmp = ln2p.tile([P, QW], f32, tag="ln2tmp", bufs=2)
            nc.vector.tensor_tensor(out=tmp, in0=x2T[dc], in1=mu_bc,
                                    op=OP.subtract)
            nc.vector.tensor_tensor(out=tmp, in0=tmp, in1=rstd_bc, op=OP.mult)
            nc.scalar.activation(out=y2T[dc], in_=tmp, func=AF.Identity,
                                 scale=ln2w_sb[:, dc : dc + 1],
                                 bias=ln2b_sb[:, dc : dc + 1])

    z_pool.release()
    xT_pool.release()

    # ---------------- Phase 7: MLP + residual + out ---------------------
    h1_pool = tc.alloc_tile_pool(name="h1_pool", bufs=1)
    h1T = [h1_pool.tile([P, QW], bf16, tag=f"h1T{fc}", name=f"h1T{fc}") for fc in range(F // P)]
    with tc.tile_pool(name="mlpw", bufs=3) as mwp, \
         tc.tile_pool(name="mlpp", bufs=3, space="PSUM") as mpp, \
         tc.tile_pool(name="mlpo", bufs=3) as mop:
        for fc in range(F // P):
            w1f = mwp.tile([P, DC, P], bf16, tag="w1f")
            nc.sync.dma_start(
                out=w1f,
                in_=w1_m[:, fc * P : (fc + 1) * P].rearrange(
                    "(dc dl) f -> dl dc f", dl=P))
            ps = mpp.tile([P, QW], f32, tag="psH")
            for dc in range(DC):
                nc.tensor.matmul(ps, w1f[:, dc, :], y2T[dc],
                                 start=(dc == 0), stop=(dc == DC - 1))
            nc.scalar.activation(out=h1T[fc], in_=ps, func=AF.Relu,
                                 bias=b1_sb[:, fc : fc + 1], scale=1.0)
        for dc in range(DC):
            w2d = mwp.tile([P, F // P, P], bf16, tag="w2d", bufs=2)
            nc.sync.dma_start(
                out=w2d,
                in_=w2_m[:, dc * P : (dc + 1) * P].rearrange(
                    "(fc fl) d -> fl fc d", fl=P))
            ps = mpp.tile([P, QW], f32, tag="psO2")
            for fc in range(F // P):
                nc.tensor.matmul(ps, w2d[:, fc, :], h1T[fc],
                                 start=(fc == 0), stop=(fc == F // P - 1))
            ot = mop.tile([P, QW], f32, tag="ot")
            nc.scalar.activation(out=ot, in_=ps, func=AF.Identity,
                                 bias=b2_sb[:, dc : dc + 1], scale=1.0)
            nc.vector.tensor_tensor(out=ot, in0=ot, in1=x2T[dc], op=OP.add)
            nc.sync.dma_start(out=out_t[dc * P : (dc + 1) * P, :], in_=ot)

    h1_pool.release()
    x2_pool.release()
    const.release()


def _core_tiles(j):
    return [j, 7 - j, 8 + j, 15 - j]


def _make_mask(j):
    import ml_dtypes

    mask = np.zeros((QT, 4, P, P), np.float32)
    g_list = _core_tiles(j)
    for i, g in enumerate(g_list):
        p_idx = 512 * i + 128 * np.arange(4)[:, None, None] + np.arange(P)[None, :, None]
        q_idx = g * 128 + np.arange(P)[None, None, :]
        mask[i] = np.where(p_idx <= q_idx, 1.0, 0.0)
    return mask.astype(ml_dtypes.bfloat16)


def _marshal(x, W_Q, W_K, W_V, W_O, ln_att_w, ln_att_b, ln_mlp_w, ln_mlp_b,
             mlp_w1, mlp_b1, mlp_w2, mlp_b2):
    import ml_dtypes

    f = np.float32
    bf = ml_dtypes.bfloat16
    com = {
        "wq_m": np.ascontiguousarray(
            (np.transpose(W_Q, (2, 0, 1)).reshape(D, H * DH)
             * (1.0 / np.sqrt(DH))).astype(bf)),
        "wk_m": np.ascontiguousarray(np.transpose(W_K, (2, 0, 1)).reshape(D, H * DH).astype(bf)),
        "wv_m": np.ascontiguousarray(np.transpose(W_V, (2, 0, 1)).reshape(D, H * DH).astype(bf)),
        "wo_m": np.ascontiguousarray(np.transpose(W_O, (2, 1, 0)).reshape(H * DH, D).astype(bf)),
        "w1_m": np.ascontiguousarray(mlp_w1.T.astype(bf)),
        "w2_m": np.ascontiguousarray(mlp_w2.T.astype(bf)),
        "ln1w": np.ascontiguousarray(ln_att_w, f),
        "ln1b": np.ascontiguousarray(ln_att_b, f),
        "ln2w": np.ascontiguousarray(ln_mlp_w, f),
        "ln2b": np.ascontiguousarray(ln_mlp_b, f),
        "b1v": np.ascontiguousarray(mlp_b1, f),
        "b2v": np.ascontiguousarray(mlp_b2, f),
    }
    masks = [_make_mask(j) for j in range(4)]
    in_maps = []
    for c in range(NCORES):
        b, j = c // 4, c % 4
        rows = np.concatenate([np.arange(g * 128, (g + 1) * 128)
                               for g in _core_tiles(j)])
        m = dict(com)
        m["x_full"] = np.ascontiguousarray(x[b], f)
        m["x_q"] = np.ascontiguousarray(x[b][rows], f)
        m["maskd"] = masks[j]
        in_maps.append(m)
    return in_maps


def _get_nc():
    if "nc" not in _CACHE:
        _CACHE["nc"] = _build_program()
    return _CACHE["nc"]


def run_on_cores(in_maps, trace=False):
    from concourse.bass_utils import run_bass_kernel_spmd

    nc = _get_nc()
    return run_bass_kernel_spmd(nc, in_maps, list(range(NCORES)), trace=trace)


def _assemble(results):
    x_out = np.empty((B, S, D), np.float32)
    att = np.empty((B, H, S, S), np.float32)
    for c in range(NCORES):
        b, j = c // 4, c % 4
        ot = results[c]["out_t"]       # [D, QW]
        at = results[c]["att_t"]       # [H, S, QW]
        for i, g in enumerate(_core_tiles(j)):
            sl = slice(g * 128, (g + 1) * 128)
            x_out[b, sl, :] = ot[:, i * 128 : (i + 1) * 128].T
            att[b, :, sl, :] = np.swapaxes(at[:, :, i * 128 : (i + 1) * 128], 1, 2)
    return x_out, att


def kernel(x, W_Q, W_K, W_V, W_O, ln_att_w, ln_att_b, ln_mlp_w, ln_mlp_b,
           mlp_w1, mlp_b1, mlp_w2, mlp_b2):
    in_maps = _marshal(x, W_Q, W_K, W_V, W_O, ln_att_w, ln_att_b,
                       ln_mlp_w, ln_mlp_b, mlp_w1, mlp_b1, mlp_w2, mlp_b2)
    res = run_on_cores(in_maps)
    return _assemble(res.results)


# revision 28
# speedup vs baseline: 1.0536x; 1.0536x over previous
"""Trainium2 Bass kernel for a pre-LN transformer decoder layer.

Shapes (hardcoded): B=2, S=2048, D=1024, H=16, DH=64, F=4096.
Returns (x_out [B,S,D], att [B,H,S,S]) like the reference.

Sharding: 8 cores; core c -> batch b=c//4, j=c%4 owns the four 128-row
query tiles {j, 7-j, 8+j, 15-j} of its batch.  Sorted by causal extent
these tiles need exactly {1,2,3,4} p-blocks of 512 keys, so every core
runs the identical (SPMD) program; which rows/mask a core gets is pure
data.  Each core computes LN1+K/V over the full 2048 rows of its batch
(duplicated across the 4 cores of a batch - keeps the program uniform
and the cores perfectly load balanced), Q/attention/W_O/MLP only for
its own 512 query rows.

Layout: activations are kept feature-on-partitions ("transposed") via
PE-transpose right after LN, so every projection is a plain
lhsT[K=feat,M] x rhs[K=feat,N=rows] matmul.  Scores are computed
transposed ([p, q]); softmax skips max-subtraction (scaled scores are
O(1); the -1.25e9 causal bias underflows exp to exactly 0) and the
denominator rides the A@V matmul as an appended ones-column of V.
Attention internals run in bf16, projections in float32r (full PE rate
at free-dim >= 256).
"""

import numpy as np

B, S, D, H, DH, F = 2, 2048, 1024, 16, 64, 4096
P = 128
DC = D // P          # 8 feature chunks
RC = S // P          # 16 row chunks per batch
RB = S // 512        # 4 row blocks of 512
QT = 4               # q tiles per core
QW = QT * P          # 512 query rows per core
HP = H // 2          # 8 head pairs
NCORES = 8
MASK_NEG = -1.25e9   # -1e10 / sqrt(DH)

_CACHE = {}


def _build_program():
    import concourse.bass as bass
    import concourse.mybir as mybir
    import concourse.tile as tile
    from concourse import bacc
    from concourse.masks import make_identity

    f32 = mybir.dt.float32
    f32r = mybir.dt.float32r
    bf16 = mybir.dt.bfloat16
    AF = mybir.ActivationFunctionType
    OP = mybir.AluOpType

    nc = bacc.Bacc(
        "TRN2", target_bir_lowering=False, debug=False, num_devices=NCORES
    )

    def din(name, shape, dt=None):
        return nc.dram_tensor(name, list(shape), dt or f32,
                              kind="ExternalInput").ap()

    def dout(name, shape):
        return nc.dram_tensor(name, list(shape), f32, kind="ExternalOutput").ap()

    x_full = din("x_full", [S, D])
    x_q = din("x_q", [QW, D])
    wq_m = din("wq_m", [D, H * DH], bf16)     # [d, h*64+dh], prescaled by 1/8
    wk_m = din("wk_m", [D, H * DH], bf16)
    wv_m = din("wv_m", [D, H * DH], bf16)
    wo_m = din("wo_m", [H * DH, D], bf16)
    w1_m = din("w1_m", [D, F], bf16)
    w2_m = din("w2_m", [F, D], bf16)
    ln1w = din("ln1w", [D])
    ln1b = din("ln1b", [D])
    ln2w = din("ln2w", [D])
    ln2b = din("ln2b", [D])
    b1v = din("b1v", [F])
    b2v = din("b2v", [D])
    maskd = din("maskd", [QT, 4, P, P], bf16)  # [i,c,p,q] 0/1 multiplicative

    att_t = nc.dram_tensor("att_t", [H, S, QW], bf16,
                           kind="ExternalOutput").ap()  # [h, p, q] bf16
    out_t = dout("out_t", [D, QW])       # transposed output [d, q]

    with tile.TileContext(nc, pool_alloc_mode="queue") as tc:
        _body(tc, nc, locals())
    nc.compile()
    return nc


def _body(tc, nc, t):
    import concourse.mybir as mybir
    from concourse.masks import make_identity

    f32 = mybir.dt.float32
    f32r = mybir.dt.float32r
    bf16 = mybir.dt.bfloat16
    AF = mybir.ActivationFunctionType
    OP = mybir.AluOpType
    X = mybir.AxisListType

    x_full, x_q = t["x_full"], t["x_q"]
    wq_m, wk_m, wv_m, wo_m = t["wq_m"], t["wk_m"], t["wv_m"], t["wo_m"]
    w1_m, w2_m = t["w1_m"], t["w2_m"]
    ln1w, ln1b, ln2w, ln2b = t["ln1w"], t["ln1b"], t["ln2w"], t["ln2b"]
    b1v, b2v, maskd = t["b1v"], t["b2v"], t["maskd"]
    att_t, out_t = t["att_t"], t["out_t"]

    r32 = lambda ap: ap.bitcast(f32r)

    const = tc.alloc_tile_pool(name="const", bufs=1)

    ident = const.tile([P, P], f32, tag="ident")
    make_identity(nc, ident)
    ones_col = const.tile([P, 1], f32, tag="ones_col")
    nc.vector.memset(ones_col, 1.0)
    ones_row = const.tile([P, P], f32, tag="ones_row")
    nc.vector.memset(ones_row, 1.0)
    eps1 = const.tile([P, 1], f32, tag="eps1")
    nc.vector.memset(eps1, 1e-5)

    def load_vec_chunks(ap_dram, name):
        # [D] -> [128, DC] (column dc = chunk dc, per-partition scalars)
        tile_ = const.tile([P, D // P], f32, tag=name)
        nc.sync.dma_start(
            out=tile_, in_=ap_dram.rearrange("(c p) -> p c", p=P)
        )
        return tile_

    ln1w_sb = load_vec_chunks(ln1w, "ln1w_sb")
    ln1b_sb = load_vec_chunks(ln1b, "ln1b_sb")
    ln2w_sb = load_vec_chunks(ln2w, "ln2w_sb")
    ln2b_sb = load_vec_chunks(ln2b, "ln2b_sb")
    b2_sb = load_vec_chunks(b2v, "b2_sb")
    b1_sb = const.tile([P, F // P], f32, tag="b1_sb")
    nc.sync.dma_start(out=b1_sb, in_=b1v.rearrange("(c p) -> p c", p=P))

    mask_sb = const.tile([P, QT, 4, P], bf16, tag="mask_sb")
    nc.sync.dma_start(out=mask_sb, in_=maskd.rearrange("i c p q -> p i c q"))

    # ---------------- Phase 1: LN1 + transpose of x_full -> yT ----------
    yT_pool = tc.alloc_tile_pool(name="yT_pool", bufs=1, side="right")
    yT = [yT_pool.tile([P, S], bf16, tag=f"yT{dc}", name=f"yT{dc}") for dc in range(DC)]

    def ln_transpose(xin_pool, stat_pool, tpsum_pool, src_dram, r0, n_yt, yt_col,
                     also_xt=None):
        # Load 128 rows starting at r0, LN them, write transposed chunks into
        # n_yt[dc][:, yt_col:yt_col+128]; optionally also raw-transpose into
        # also_xt[dc][:, ...].
        xt_ = xin_pool.tile([P, D], f32, tag="x_in")
        nc.sync.dma_start(out=xt_, in_=src_dram[r0 : r0 + P, :])
        stats = stat_pool.tile([P, 2, 6], f32, tag="stats")
        for sg in range(2):
            nc.vector.bn_stats(out=stats[:, sg, :], in_=xt_[:, sg * 512 : (sg + 1) * 512])
        mv = stat_pool.tile([P, 2], f32, tag="mv")
        nc.vector.bn_aggr(out=mv, in_=stats)
        rstd = stat_pool.tile([P, 1], f32, tag="rstd")
        nc.scalar.activation(out=rstd, in_=mv[:, 1:2], func=AF.Sqrt,
                             bias=eps1, scale=1.0)
        nc.vector.reciprocal(out=rstd, in_=rstd)
        n_t = xin_pool.tile([P, D], f32, tag="n_t")
        nc.vector.tensor_scalar(out=n_t, in0=xt_, scalar1=mv[:, 0:1],
                                scalar2=rstd, op0=OP.subtract, op1=OP.mult)
        for dc in range(DC):
            tp = tpsum_pool.tile([P, P], f32, tag="tp")
            nc.tensor.transpose(tp, n_t[:, dc * P : (dc + 1) * P], ident)
            nc.scalar.activation(
                out=n_yt[dc][:, yt_col : yt_col + P], in_=tp, func=AF.Identity,
                scale=ln1w_sb[:, dc : dc + 1], bias=ln1b_sb[:, dc : dc + 1])
            if also_xt is not None:
                tp2 = tpsum_pool.tile([P, P], f32, tag="tp2")
                nc.tensor.transpose(tp2, xt_[:, dc * P : (dc + 1) * P], ident)
                nc.vector.tensor_copy(out=also_xt[dc][:, yt_col : yt_col + P],
                                      in_=tp2)

    with tc.tile_pool(name="ph1", bufs=3) as xin_pool, \
         tc.tile_pool(name="ph1s", bufs=4) as stat_pool, \
         tc.tile_pool(name="ph1p", bufs=4, space="PSUM") as tpsum_pool:
        for rc in range(RC):
            ln_transpose(xin_pool, stat_pool, tpsum_pool, x_full, rc * P,
                         yT, rc * P)

    # ---------------- Phase 2: K projection -> kT (bf16) ----------------
    attin_pool = tc.alloc_tile_pool(name="attin_pool", bufs=1)
    kT = [attin_pool.tile([P, S], bf16, tag=f"kT{hp}", name=f"kT{hp}") for hp in range(HP)]
    with tc.tile_pool(name="wk", bufs=1) as wkp, \
         tc.tile_pool(name="kps", bufs=4, space="PSUM") as kps:
        wk_sb = [wkp.tile([P, H * DH], bf16, tag=f"wk{dc}", name=f"wk{dc}") for dc in range(DC)]
        for dc in range(DC):
            nc.sync.dma_start(out=wk_sb[dc], in_=wk_m[dc * P : (dc + 1) * P, :])
        for hp in range(HP):
            for rb in range(RB):
                ps = kps.tile([P, 512], f32, tag="psK")
                for dc in range(DC):
                    nc.tensor.matmul(
                        ps, wk_sb[dc][:, hp * P : (hp + 1) * P],
                        yT[dc][:, rb * 512 : (rb + 1) * 512],
                        start=(dc == 0), stop=(dc == DC - 1))
                nc.scalar.activation(out=kT[hp][:, rb * 512 : (rb + 1) * 512],
                                     in_=ps, func=AF.Copy)

    # ---------------- Phase 3: V projection -> v_sb (bf16, ones col) ----
    # v_sb[rc] is [128, H*65]; column h*65+64 holds ones (denominator trick)
    v_sb = [attin_pool.tile([P, H * 65], bf16, tag=f"v{rc}", name=f"v{rc}") for rc in range(RC)]
    with tc.tile_pool(name="wv", bufs=1) as wvp, \
         tc.tile_pool(name="vps", bufs=4, space="PSUM") as vps:
        wv_sb = [wvp.tile([P, H * DH], bf16, tag=f"wv{dc}", name=f"wv{dc}") for dc in range(DC)]
        for dc in range(DC):
            nc.sync.dma_start(out=wv_sb[dc], in_=wv_m[dc * P : (dc + 1) * P, :])
        for rc in range(RC):
            nc.vector.memset(v_sb[rc], 1.0)
            for hg in range(2):
                ps = vps.tile([P, 512], f32, tag="psV")
                for dc in range(DC):
                    nc.tensor.matmul(
                        ps, yT[dc][:, rc * P : (rc + 1) * P],
                        wv_sb[dc][:, hg * 512 : (hg + 1) * 512],
                        start=(dc == 0), stop=(dc == DC - 1))
                vdst = v_sb[rc].rearrange("p (h e) -> p h e", e=65)
                nc.vector.tensor_copy(
                    out=vdst[:, hg * 8 : (hg + 1) * 8, 0:64],
                    in_=ps.rearrange("p (h e) -> p h e", e=64))

    yT_pool.release()

    # ---------------- Phase 4: x_q LN + transposes, Q projection --------
    xT_pool = tc.alloc_tile_pool(name="xT_pool", bufs=1, side="right")
    yqT_pool = tc.alloc_tile_pool(name="yqT_pool", bufs=1, side="right")
    yqT = [yqT_pool.tile([P, QW], bf16, tag=f"yqT{dc}", name=f"yqT{dc}") for dc in range(DC)]
    xT = [xT_pool.tile([P, QW], f32, tag=f"xT{dc}", name=f"xT{dc}") for dc in range(DC)]
    with tc.tile_pool(name="ph4", bufs=3) as xin_pool, \
         tc.tile_pool(name="ph4s", bufs=4) as stat_pool, \
         tc.tile_pool(name="ph4p", bufs=3, space="PSUM") as tpsum_pool:
        for qt in range(QT):
            ln_transpose(xin_pool, stat_pool, tpsum_pool, x_q, qt * P,
                         yqT, qt * P, also_xt=xT)

    qT = [attin_pool.tile([P, QW], bf16, tag=f"qT{hp}", name=f"qT{hp}") for hp in range(HP)]
    with tc.tile_pool(name="wq", bufs=1) as wqp, \
         tc.tile_pool(name="qps", bufs=4, space="PSUM") as qps:
        wq_sb = [wqp.tile([P, H * DH], bf16, tag=f"wq{dc}", name=f"wq{dc}") for dc in range(DC)]
        for dc in range(DC):
            nc.sync.dma_start(out=wq_sb[dc], in_=wq_m[dc * P : (dc + 1) * P, :])
        for hp in range(HP):
            ps = qps.tile([P, QW], f32, tag="psQ")
            for dc in range(DC):
                nc.tensor.matmul(ps, wq_sb[dc][:, hp * P : (hp + 1) * P],
                                 yqT[dc], start=(dc == 0),
                                 stop=(dc == DC - 1))
            nc.scalar.activation(out=qT[hp], in_=ps, func=AF.Copy)

    yqT_pool.release()

    # ---------------- Phase 5: attention ------------------------------
    z_pool = tc.alloc_tile_pool(name="z_pool", bufs=1, side="right")
    z_sb = [z_pool.tile([P, QW], bf16, tag=f"z{hp}", name=f"z{hp}") for hp in range(HP)]
    with tc.tile_pool(name="att_sp", bufs=3, space="PSUM") as spp, \
         tc.tile_pool(name="att_zp", bufs=2, space="PSUM") as zpp, \
         tc.tile_pool(name="att_exp", bufs=4) as expp, \
         tc.tile_pool(name="att_sm", bufs=3) as smp, \
         tc.tile_pool(name="att_dscr", bufs=3, space="DRAM") as dscr, \
         tc.tile_pool(name="att_st0", bufs=2) as attst0, \
         tc.tile_pool(name="att_st1", bufs=2) as attst1, \
         tc.tile_pool(name="att_st2", bufs=2) as attst2, \
         tc.tile_pool(name="att_st3", bufs=2) as attst3:
        attst = [attst0, attst1, attst2, attst3]
        for h in range(H):
            hp, hl = h // 2, h % 2
            base = hl * 64
            exps = []
            for i in range(QT):
                qcols = QW - 128 * i
                et = expp.tile([P, 4, qcols], bf16, tag=f"exp{i}", name=f"exp_h{h}_{i}")
                exps.append(et)
                if i < 2:
                    # one psum tile + exp per 128-p chunk
                    for c in range(4):
                        ps = spp.tile([P, qcols], f32, tag=f"sp{hl}")
                        nc.tensor.matmul(
                            ps,
                            kT[hp][base : base + 64,
                                   512 * i + 128 * c : 512 * i + 128 * (c + 1)],
                            qT[hp][base : base + 64, 128 * i : QW],
                            start=True, stop=True)
                        nc.scalar.activation(out=et[:, c, :], in_=ps,
                                             func=AF.Exp)
                else:
                    # pack chunks into one psum bank, single exp per group
                    ng = 2 if i == 2 else 4
                    for g in range(4 // ng):
                        ps = spp.tile([P, ng, qcols], f32, tag=f"sp{hl}")
                        for cc in range(ng):
                            c = g * ng + cc
                            nc.tensor.matmul(
                                ps[:, cc, :],
                                kT[hp][base : base + 64,
                                       512 * i + 128 * c : 512 * i + 128 * (c + 1)],
                                qT[hp][base : base + 64, 128 * i : QW],
                                start=True, stop=True)
                        nc.scalar.activation(
                            out=et[:, g * ng : (g + 1) * ng, :], in_=ps,
                            func=AF.Exp)
                # causal mask for this q-tile's own columns (batched, 4 chunks)
                nc.vector.tensor_tensor(
                    out=et[:, :, 0:P], in0=et[:, :, 0:P],
                    in1=mask_sb[:, i, :, :], op=OP.mult)
            zb = zpp.tile([65, QW], f32, tag="zb")
            for i in range(QT):
                for c in range(4):
                    nc.tensor.matmul(
                        zb[0:65, 128 * i : QW],
                        v_sb[4 * i + c][:, h * 65 : h * 65 + 65],
                        exps[i][:, c, :],
                        start=(i == 0 and c == 0),
                        stop=(i == QT - 1 and c == 3),
                        skip_group_check=True)
            rec64 = smp.tile([P, QW], bf16, tag="rec64")
            with nc.allow_low_precision(reason="bf16 attention path"):
                nc.vector.reciprocal(out=rec64[64:65, :], in_=zb[64:65, :])
            rec_d = dscr.tile([1, QW], bf16, tag="rec_d")
            nc.sync.dma_start(out=rec_d, in_=rec64[64:65, :])
            rec_bc = smp.tile([P, QW], bf16, tag="rec_bc")
            nc.sync.dma_start(out=rec_bc, in_=rec_d.to_broadcast([P, QW]))
            # z rows for this head: normalize straight out of PSUM
            zt = smp.tile([64, QW], bf16, tag="zt")
            nc.vector.tensor_tensor(out=zt, in0=zb[0:64, :],
                                    in1=rec_bc[0:64, :], op=OP.mult)
            nc.sync.dma_start(out=z_sb[hp][base : base + 64, :], in_=zt)
            # normalized attention out (f32), one batched mul + DMA per i
            for i in range(QT):
                qcols = QW - 128 * i
                af = attst[i].tile([P, 4, qcols], bf16, tag=f"af{i}",
                                   name=f"af{i}")
                eng = nc.vector if i < 2 else nc.gpsimd
                eng.tensor_tensor(
                    out=af, in0=exps[i],
                    in1=rec_bc[:, 128 * i : QW].unsqueeze(1).to_broadcast(
                        [P, 4, qcols]),
                    op=OP.mult)
                nc.sync.dma_start(
                    out=att_t[h, 512 * i : 512 * (i + 1),
                              128 * i : QW].rearrange("(c p) q -> p c q", p=P),
                    in_=af)

    attin_pool.release()

    # ---------------- Phase 6: W_O, residual, LN2 -----------------------
    x2_pool = tc.alloc_tile_pool(name="x2_pool", bufs=1)
    x2T = [x2_pool.tile([P, QW], f32, tag=f"x2T{dc}", name=f"x2T{dc}") for dc in range(DC)]
    y2T = [x2_pool.tile([P, QW], bf16, tag=f"y2T{dc}", name=f"y2T{dc}") for dc in range(DC)]
    with tc.tile_pool(name="wo", bufs=1) as wop, \
         tc.tile_pool(name="ops", bufs=4, space="PSUM") as ops_, \
         tc.tile_pool(name="ln2", bufs=1) as ln2p:
        wo_sb = [wop.tile([P, D], bf16, tag=f"wo{kc}", name=f"wo{kc}") for kc in range(DC)]
        for kc in range(DC):
            nc.sync.dma_start(out=wo_sb[kc], in_=wo_m[kc * P : (kc + 1) * P, :])
        sq = [ln2p.tile([P, QW], f32, tag=f"sq{dc % 4}", name=f"sq{dc}") for dc in range(DC)]
        for dc in range(DC):
            ps = ops_.tile([P, QW], f32, tag="psO")
            for kc in range(DC):
                nc.tensor.matmul(ps, wo_sb[kc][:, dc * P : (dc + 1) * P],
                                 z_sb[kc], start=(kc == 0),
                                 stop=(kc == DC - 1))
            nc.vector.tensor_tensor(out=x2T[dc], in0=ps, in1=xT[dc], op=OP.add)
            nc.vector.tensor_tensor(out=sq[dc], in0=x2T[dc], in1=x2T[dc],
                                    op=OP.mult)
        ps1 = ops_.tile([1, QW], f32, tag="ps1", bufs=1)
        ps2 = ops_.tile([1, QW], f32, tag="ps2", bufs=1)
        for dc in range(DC):
            nc.tensor.matmul(ps1, ones_col, x2T[dc],
                             start=(dc == 0), stop=(dc == DC - 1))
        for dc in range(DC):
            nc.tensor.matmul(ps2, ones_col, sq[dc],
                             start=(dc == 0), stop=(dc == DC - 1))
        mu = ln2p.tile([1, QW], f32, tag="mu")
        nc.scalar.mul(out=mu, in_=ps1, mul=1.0 / D)
        ex2 = ln2p.tile([1, QW], f32, tag="ex2")
        nc.scalar.mul(out=ex2, in_=ps2, mul=1.0 / D)
        var = ln2p.tile([1, QW], f32, tag="var")
        nc.vector.tensor_tensor(out=var, in0=mu, in1=mu, op=OP.mult)
        nc.vector.tensor_tensor(out=var, in0=ex2, in1=var, op=OP.subtract)
        rstd2 = ln2p.tile([1, QW], f32, tag="rstd2")
        nc.scalar.activation(out=rstd2, in_=var, func=AF.Sqrt,
                             bias=eps1[0:1, :], scale=1.0)
        nc.vector.reciprocal(out=rstd2, in_=rstd2)
        mu_ps = ops_.tile([P, QW], f32, tag="mu_ps", bufs=1)
        nc.tensor.matmul(mu_ps, ones_row[0:1, :], mu, start=True, stop=True)
        mu_bc = ln2p.tile([P, QW], f32, tag="mu_bc")
        nc.scalar.activation(out=mu_bc, in_=mu_ps, func=AF.Copy)
        rstd_ps = ops_.tile([P, QW], f32, tag="rstd_ps", bufs=1)
        nc.tensor.matmul(rstd_ps, ones_row[0:1, :], rstd2, start=True, stop=True)
        rstd_bc = ln2p.tile([P, QW], f32, tag="rstd_bc")
        nc.scalar.activation(out=rstd_bc, in_=rstd_ps, func=AF.Copy)
        for dc in range(DC):
            tmp = ln2p.tile([P, QW], f32, tag="ln2tmp", bufs=2)
            nc.vector.tensor_tensor(out=tmp, in0=x2T[dc], in1=mu_bc,
                                    op=OP.subtract)
            nc.vector.tensor_tensor(out=tmp, in0=tmp, in1=rstd_bc, op=OP.mult)
            nc.scalar.activation(out=y2T[dc], in_=tmp, func=AF.Identity,
                                 scale=ln2w_sb[:, dc : dc + 1],
                                 bias=ln2b_sb[:, dc : dc + 1])

    z_pool.release()
    xT_pool.release()

    # ---------------- Phase 7: MLP + residual + out ---------------------
    h1_pool = tc.alloc_tile_pool(name="h1_pool", bufs=1)
    h1T = [h1_pool.tile([P, QW], bf16, tag=f"h1T{fc}", name=f"h1T{fc}") for fc in range(F // P)]
    with tc.tile_pool(name="mlpw", bufs=3) as mwp, \
         tc.tile_pool(name="mlpp", bufs=3, space="PSUM") as mpp, \
         tc.tile_pool(name="mlpo", bufs=3) as mop:
        for fc in range(F // P):
            w1f = mwp.tile([P, DC, P], bf16, tag="w1f")
            nc.sync.dma_start(
                out=w1f,
                in_=w1_m[:, fc * P : (fc + 1) * P].rearrange(
                    "(dc dl) f -> dl dc f", dl=P))
            ps = mpp.tile([P, QW], f32, tag="psH")
            for dc in range(DC):
                nc.tensor.matmul(ps, w1f[:, dc, :], y2T[dc],
                                 start=(dc == 0), stop=(dc == DC - 1))
            nc.scalar.activation(out=h1T[fc], in_=ps, func=AF.Relu,
                                 bias=b1_sb[:, fc : fc + 1], scale=1.0)
        for dc in range(DC):
            w2d = mwp.tile([P, F // P, P], bf16, tag="w2d", bufs=2)
            nc.sync.dma_start(
                out=w2d,
                in_=w2_m[:, dc * P : (dc + 1) * P].rearrange(
                    "(fc fl) d -> fl fc d", fl=P))
            ps = mpp.tile([P, QW], f32, tag="psO2")
            for fc in range(F // P):
                nc.tensor.matmul(ps, w2d[:, fc, :], h1T[fc],
                                 start=(fc == 0), stop=(fc == F // P - 1))
            ot = mop.tile([P, QW], f32, tag="ot")
            nc.scalar.activation(out=ot, in_=ps, func=AF.Identity,
                                 bias=b2_sb[:, dc : dc + 1], scale=1.0)
            nc.vector.tensor_tensor(out=ot, in0=ot, in1=x2T[dc], op=OP.add)
            nc.sync.dma_start(out=out_t[dc * P : (dc + 1) * P, :], in_=ot)

    h1_pool.release()
    x2_pool.release()
    const.release()


def _core_tiles(j):
    return [j, 7 - j, 8 + j, 15 - j]


def _make_mask(j):
    import ml_dtypes

    mask = np.zeros((QT, 4, P, P), np.float32)
    g_list = _core_tiles(j)
    for i, g in enumerate(g_list):
        p_idx = 512 * i + 128 * np.arange(4)[:, None, None] + np.arange(P)[None, :, None]
        q_idx = g * 128 + np.arange(P)[None, None, :]
        mask[i] = np.where(p_idx <= q_idx, 1.0, 0.0)
    return mask.astype(ml_dtypes.bfloat16)


def _marshal(x, W_Q, W_K, W_V, W_O, ln_att_w, ln_att_b, ln_mlp_w, ln_mlp_b,
             mlp_w1, mlp_b1, mlp_w2, mlp_b2):
    import ml_dtypes

    f = np.float32
    bf = ml_dtypes.bfloat16
    com = {
        "wq_m": np.ascontiguousarray(
            (np.transpose(W_Q, (2, 0, 1)).reshape(D, H * DH)
             * (1.0 / np.sqrt(DH))).astype(bf)),
        "wk_m": np.ascontiguousarray(np.transpose(W_K, (2, 0, 1)).reshape(D, H * DH).astype(bf)),
        "wv_m": np.ascontiguousarray(np.transpose(W_V, (2, 0, 1)).reshape(D, H * DH).astype(bf)),
        "wo_m": np.ascontiguousarray(np.transpose(W_O, (2, 1, 0)).reshape(H * DH, D).astype(bf)),
        "w1_m": np.ascontiguousarray(mlp_w1.T.astype(bf)),
        "w2_m": np.ascontiguousarray(mlp_w2.T.astype(bf)),
        "ln1w": np.ascontiguousarray(ln_att_w, f),
        "ln1b": np.ascontiguousarray(ln_att_b, f),
        "ln2w": np.ascontiguousarray(ln_mlp_w, f),
        "ln2b": np.ascontiguousarray(ln_mlp_b, f),
        "b1v": np.ascontiguousarray(mlp_b1, f),
        "b2v": np.ascontiguousarray(mlp_b2, f),
    }
    masks = [_make_mask(j) for j in range(4)]
    in_maps = []
    for c in range(NCORES):
        b, j = c // 4, c % 4
        rows = np.concatenate([np.arange(g * 128, (g + 1) * 128)
                               for g in _core_tiles(j)])
        m = dict(com)
        m["x_full"] = np.ascontiguousarray(x[b], f)
        m["x_q"] = np.ascontiguousarray(x[b][rows], f)
        m["maskd"] = masks[j]
        in_maps.append(m)
    return in_maps


def _get_nc():
    if "nc" not in _CACHE:
        _CACHE["nc"] = _build_program()
    return _CACHE["nc"]


def run_on_cores(in_maps, trace=False):
    from concourse.bass_utils import run_bass_kernel_spmd

    nc = _get_nc()
    return run_bass_kernel_spmd(nc, in_maps, list(range(NCORES)), trace=trace)


def _assemble(results):
    x_out = np.empty((B, S, D), np.float32)
    att = np.empty((B, H, S, S), np.float32)
    for c in range(NCORES):
        b, j = c // 4, c % 4
        ot = results[c]["out_t"]       # [D, QW]
        at = results[c]["att_t"]       # [H, S, QW]
        for i, g in enumerate(_core_tiles(j)):
            sl = slice(g * 128, (g + 1) * 128)
            x_out[b, sl, :] = ot[:, i * 128 : (i + 1) * 128].T
            att[b, :, sl, :] = np.swapaxes(at[:, :, i * 128 : (i + 1) * 128], 1, 2)
    return x_out, att


def kernel(x, W_Q, W_K, W_V, W_O, ln_att_w, ln_att_b, ln_mlp_w, ln_mlp_b,
           mlp_w1, mlp_b1, mlp_w2, mlp_b2):
    in_maps = _marshal(x, W_Q, W_K, W_V, W_O, ln_att_w, ln_att_b,
                       ln_mlp_w, ln_mlp_b, mlp_w1, mlp_b1, mlp_w2, mlp_b2)
    res = run_on_cores(in_maps)
    return _assemble(res.results)
